# revision 23
# baseline (speedup 1.0000x reference)
"""Trainium2 Bass kernel for a transformer encoder layer (B=4, S=2048,
D=1024, H=16, DFF=4096, fp32).

Sharding: 8 cores = (batch b, query-half). Each core computes K/V for its
full batch (replicated within the pair) and Q/attention/FFN for its 1024
query tokens. No collectives. Host reorders tokens so each core's queries
are tokens 0..1023 of its (permutation-invariant) key set.

Per-core dataflow (layouts chosen so every matmul contracts along the
partition dim; the only on-chip transpose is out1 -> out1T):
  xT [D, S] host-transposed bf16
  QT/KT depth-major bf16 (head pairs packed 64+64 in partitions)
  V [tok, head, 65] bf16 with a ones column -> softmax denominator for free
  S^T = KT.T @ QT per (head, kc); exp on ACT (mask + 1/8 scale fused) -> PT bf16
  ctxT+denom = V_aug.T @ PT ; normalize via 1/denom broadcast ; ctxT bf16
  attn_out = ctxT.T @ wo (bf16) ; +x (+bo host-folded) ; LN1
  PE-transpose out1 -> out1T f32r ; FFN1 = relu(w1.T @ out1T + b1) -> hT f32r
  FFN2 = hT.T @ w2 (+b2 via K=1 ones-row matmul) ; +out1 ; LN2 -> out

Projection and attention emission is interleaved in two halves so the
scalar engine starts softmax exps while the PE is still projecting.
"""

import sys

sys.path.insert(0, "/opt/trn_rl_repo")

import numpy as np
import ml_dtypes

import concourse.bass as bass
import concourse.tile as tile
from concourse import bacc, mybir
from concourse.bass_utils import run_bass_kernel_spmd
from concourse.masks import make_identity

P = 128
D = 1024
S = 2048
TQ = 1024  # query tokens per core
H = 16
DEP = 64
DFF = 4096
F32 = mybir.dt.float32
F32R = mybir.dt.float32r
BF16 = mybir.dt.bfloat16
AF = mybir.ActivationFunctionType
ALU = mybir.AluOpType
EPS = 1e-6


def build():
    nc = bacc.Bacc("TRN2", target_bir_lowering=False)

    # ---- DRAM I/O ----
    xt = nc.dram_tensor("xt", [D, S], BF16, kind="ExternalInput")
    xq = nc.dram_tensor("xq", [TQ, D], F32, kind="ExternalInput")  # +bo folded
    maskb = nc.dram_tensor("maskb", [S], F32, kind="ExternalInput")
    wq = nc.dram_tensor("wq", [D, D], BF16, kind="ExternalInput")
    wk = nc.dram_tensor("wk", [D, D], BF16, kind="ExternalInput")
    wv = nc.dram_tensor("wv", [D, D], BF16, kind="ExternalInput")
    bq = nc.dram_tensor("bq", [D], F32, kind="ExternalInput")
    bk = nc.dram_tensor("bk", [D], F32, kind="ExternalInput")
    bv = nc.dram_tensor("bv", [D], F32, kind="ExternalInput")
    wo = nc.dram_tensor("wo", [D, D], BF16, kind="ExternalInput")
    w1p = nc.dram_tensor("w1p", [DFF // P, P, D // P, P], F32R, kind="ExternalInput")
    b1 = nc.dram_tensor("b1", [DFF], F32, kind="ExternalInput")
    w2 = nc.dram_tensor("w2", [DFF, D], F32R, kind="ExternalInput")
    b2 = nc.dram_tensor("b2", [D], F32R, kind="ExternalInput")
    g1 = nc.dram_tensor("g1", [D], F32, kind="ExternalInput")
    be1 = nc.dram_tensor("be1", [D], F32, kind="ExternalInput")
    g2 = nc.dram_tensor("g2", [D], F32, kind="ExternalInput")
    be2 = nc.dram_tensor("be2", [D], F32, kind="ExternalInput")
    out = nc.dram_tensor("out", [TQ, D], F32, kind="ExternalOutput")
    warm_dram = nc.dram_tensor("warm_scratch", [P, 16], F32, kind="Internal")
    recip_dram = nc.dram_tensor("recip_scratch", [H, TQ], F32, kind="Internal")

    def bcast(handle, n):
        return bass.AP(tensor=handle, offset=0, ap=[[0, P], [1, n]])

    with tile.TileContext(nc) as tc:
        consts = tc.alloc_tile_pool(name="consts", bufs=1)

        mask_bias = consts.tile([P, S // P], F32)  # mask[k]*-1e9, k = kc*128+p
        nc.gpsimd.dma_start(out=mask_bias,
                            in_=maskb.ap().rearrange("(kc p) -> p kc", p=P))
        nc.vector.tensor_scalar_mul(out=mask_bias, in0=mask_bias, scalar1=-1e9)

        bq_sb = consts.tile([P, D // P], F32)
        bk_sb = consts.tile([P, D // P], F32)
        nc.gpsimd.dma_start(out=bq_sb, in_=bq.ap().rearrange("(m p) -> p m", p=P))
        nc.gpsimd.dma_start(out=bk_sb, in_=bk.ap().rearrange("(m p) -> p m", p=P))
        bv_bc = consts.tile([P, D], F32)
        nc.gpsimd.dma_start(out=bv_bc, in_=bcast(bv, D))
        b1_sb = consts.tile([P, DFF // P], F32)
        nc.gpsimd.dma_start(out=b1_sb, in_=b1.ap().rearrange("(m p) -> p m", p=P))
        b2_sb = consts.tile([1, D], F32R)
        nc.gpsimd.dma_start(out=b2_sb, in_=bass.AP(tensor=b2, offset=0, ap=[[0, 1], [1, D]]))
        ones_row = consts.tile([1, P], F32R)
        ones_mat = consts.tile([P, P], F32, tag="ones_mat")
        nc.vector.memset(ones_mat, 1.0)
        nc.vector.tensor_copy(out=ones_row, in_=ones_mat[0:1])
        ident = consts.tile([P, P], F32)
        make_identity(nc, ident)
        eps_t = consts.tile([P, 1], F32)
        nc.vector.memset(eps_t, EPS)

        # ---------- persistent activation tensors (split per half for
        # clean producer/consumer separation in the interleaved schedule) ----
        qkv_pool = tc.alloc_tile_pool(name="qkv", bufs=1)
        # depth-major Q/K: partition = (h%2)*64 + d, plane = h//2 - 4*half
        QTh = [qkv_pool.tile([P, 4, TQ], BF16, name=f"QT{i}") for i in range(2)]
        KTh = [qkv_pool.tile([P, 4, S], BF16, name=f"KT{i}") for i in range(2)]
        # V: [tok-part, tok-chunk, head - 8*half, 65]; col 64 = ones
        Vh = [qkv_pool.tile([P, S // P, 8, DEP + 1], BF16, name=f"V{i}")
              for i in range(2)]

        # ---------- attention output (right stack: outlives qkv) ----------
        ctx_pool = tc.alloc_tile_pool(name="ctx", bufs=1, side="right")
        ctxT = ctx_pool.tile([P, D // P, TQ], BF16)  # head h at part (h%2)*64
        denom = ctx_pool.tile([H, TQ], F32)

        # ---------- PE warm-up: a few fp32 matmuls so HAM un-throttles ----
        with tc.tile_pool(name="warm", bufs=1) as wp, \
             tc.tile_pool(name="warm_ps", bufs=1, space="PSUM") as wps:
            wjunk = wp.tile([P, 16], F32)
            wpt = wps.tile([P, P], F32)
            for i in range(10):
                nc.tensor.matmul(wpt, ident, ident, start=(i == 0),
                                 stop=(i == 9))
            nc.vector.tensor_copy(out=wjunk, in_=wpt[:, 0:16])
            nc.sync.dma_start(out=warm_dram.ap(), in_=wjunk)

        # ================= interleaved projections + attention ============
        with tc.tile_pool(name="phA", bufs=1) as pa, \
             tc.tile_pool(name="phA_wqk", bufs=3) as paw, \
             tc.tile_pool(name="phA_wv", bufs=1) as pawv, \
             tc.tile_pool(name="phB_pt", bufs=1) as ptp, \
             tc.tile_pool(name="phB_misc", bufs=2) as pbm, \
             tc.tile_pool(name="psA", bufs=2, space="PSUM") as psA, \
             tc.tile_pool(name="psB_st", bufs=2, space="PSUM") as psST, \
             tc.tile_pool(name="psB_ctx", bufs=2, space="PSUM") as psCTX:
            xt_r = xt.ap().rearrange("(ko p) t -> p ko t", p=P)
            for half in range(2):
                for t in range(S // P):
                    nc.gpsimd.memset(Vh[half][:, t, :, DEP:DEP + 1], 1.0)

            wq_r = wq.ap().rearrange("(ko p) n -> p ko n", p=P)
            wk_r = wk.ap().rearrange("(ko p) n -> p ko n", p=P)
            wv_r = wv.ap().rearrange("(ko p) n -> p ko n", p=P)

            def project_qk(xt_sb, w_r, b_sb, Tt, ntok, m):
                # lhsT = w chunk [128din, 128dout]; rhs = xT
                wt = paw.tile([P, D // P, P], BF16, tag="wqk", name="wqk")
                nc.sync.dma_start(out=wt, in_=w_r[:, :, m * P:(m + 1) * P])
                for qc in range(ntok // 512):
                    ps = psA.tile([P, 512], F32, tag="psA", name="psA")
                    for ki in range(D // P):
                        nc.tensor.matmul(ps, wt[:, ki],
                                         xt_sb[:, ki, qc * 512:(qc + 1) * 512],
                                         start=(ki == 0), stop=(ki == D // P - 1))
                    nc.vector.tensor_scalar(
                        out=Tt[:, m % 4, qc * 512:(qc + 1) * 512], in0=ps,
                        scalar1=b_sb[:, m:m + 1], scalar2=None, op0=ALU.add)

            def project_v(xt_sb, ah):
                wvt = pawv.tile([P, D // P, 512], BF16, tag="wv", name="wv")
                nc.sync.dma_start(out=wvt,
                                  in_=wv_r[:, :, ah * 512:(ah + 1) * 512])
                for t in range(S // P):
                    ps = psA.tile([P, 512], F32, tag="psA", name="psA")
                    for ki in range(D // P):
                        nc.tensor.matmul(ps, xt_sb[:, ki, t * P:(t + 1) * P],
                                         wvt[:, ki],
                                         start=(ki == 0), stop=(ki == D // P - 1))
                    nc.vector.scalar_tensor_tensor(
                        out=Vh[ah][:, t, :, 0:DEP],
                        in0=ps.rearrange("p (h d) -> p h d", h=8),
                        scalar=1.0, op0=ALU.bypass, op1=ALU.add,
                        in1=bv_bc[:, ah * 512:(ah + 1) * 512].rearrange(
                            "p (h d) -> p h d", h=8))

            def attention(hp, qc):
                half, hpl = hp // 4, hp % 4
                QT, KT, V = QTh[half], KTh[half], Vh[half]
                pt = ptp.tile([P, S // P, 2, 512], BF16, tag="pt", name="pt")
                for kc in range(S // P):
                    st = psST.tile([P, 2, 512], F32, tag="st", name="st")
                    for e in range(2):
                        nc.tensor.matmul(
                            st[:, e],
                            KT[e * DEP:(e + 1) * DEP, hpl, kc * P:(kc + 1) * P],
                            QT[e * DEP:(e + 1) * DEP, hpl, qc * 512:(qc + 1) * 512],
                            start=True, stop=True)
                    nc.scalar.activation(
                        out=pt[:, kc], in_=st, func=AF.Exp,
                        bias=mask_bias[:, kc:kc + 1], scale=0.125)
                qsl = slice(qc * 512, (qc + 1) * 512)
                for e in range(2):
                    h = hp * 2 + e
                    cps = psCTX.tile([P, 512], F32, tag="cps", name="cps")
                    for kc in range(S // P):
                        nc.tensor.matmul(
                            cps[0:DEP + 1], V[:, kc, 2 * hpl + e, :],
                            pt[:, kc, e],
                            start=(kc == 0), stop=(kc == S // P - 1))
                    stg = pbm.tile([DEP + 1, 512], F32, tag="stg", name="stg")
                    nc.vector.tensor_copy(out=stg, in_=cps[0:DEP + 1])
                    # rows 0..63 -> ctxT (bf16, cast in DMA); row 64 = denom
                    prt = slice((h % 2) * DEP, (h % 2) * DEP + DEP)
                    nc.gpsimd.dma_start(out=ctxT[prt, hp, qsl], in_=stg[0:DEP])
                    nc.sync.dma_start(out=denom[h:h + 1, qsl],
                                      in_=stg[DEP:DEP + 1])

            xt_sb = pa.tile([P, D // P, S], BF16, name="xt_sb")
            for ki in range(D // P):
                nc.sync.dma_start(out=xt_sb[:, ki], in_=xt_r[:, ki])

            # flat interleave: feed one producer unit between attention
            # chunks so the scalar engine (exp) never starves.
            # build explicit schedule
            def u_v(a):
                return lambda: project_v(xt_sb, a)

            def u_qk(m):
                def f():
                    project_qk(xt_sb, wq_r, bq_sb, QTh[m // 4], TQ, m)
                    project_qk(xt_sb, wk_r, bk_sb, KTh[m // 4], S, m)
                return f

            def u_attn(hp, qc, i):
                return lambda: attention(hp, qc)

            sched = [u_v(0), u_qk(0),
                     u_attn(0, 0, 0), u_qk(1), u_attn(0, 1, 1), u_qk(2),
                     u_attn(1, 0, 2), u_qk(3), u_attn(1, 1, 3), u_v(1),
                     u_attn(2, 0, 4), u_qk(4), u_attn(2, 1, 5), u_qk(5),
                     u_attn(3, 0, 6), u_qk(6), u_attn(3, 1, 7), u_qk(7),
                     u_attn(4, 0, 8), u_attn(4, 1, 9),
                     u_attn(5, 0, 10), u_attn(5, 1, 11),
                     u_attn(6, 0, 12), u_attn(6, 1, 13),
                     u_attn(7, 0, 14), u_attn(7, 1, 15)]
            for unit in sched:
                unit()

            # softmax normalization: ctxT *= 1/denom (per head, per q)
            nc.vector.reciprocal_approx_fast(out=denom, in_=denom)
            nc.sync.dma_start(out=recip_dram.ap(), in_=denom)
            for h in range(H):
                for qc in range(TQ // 512):
                    rb = pbm.tile([P, 512], F32, tag="rb", name="rb")
                    prt = slice((h % 2) * DEP, (h % 2) * DEP + DEP)
                    qsl = slice(qc * 512, (qc + 1) * 512)
                    nc.sync.dma_start(
                        out=rb[prt],
                        in_=recip_dram.ap()[h:h + 1, qsl].partition_broadcast(
                            DEP).squeeze(1))
                    eng = nc.vector if (h + qc) % 2 == 0 else nc.gpsimd
                    eng.tensor_tensor(
                        out=ctxT[prt, h // 2, qsl], in0=ctxT[prt, h // 2, qsl],
                        in1=rb[prt], op=ALU.mult)

        qkv_pool.release()

        # ======== wo + residual + LN1 + transpose ========
        ffnin = tc.alloc_tile_pool(name="ffnin", bufs=1)
        out1 = ffnin.tile([P, TQ // P, D], F32)    # token qm*128+p
        out1T = ffnin.tile([P, D // P, TQ], F32R)  # d dm*128+p

        with tc.tile_pool(name="phC_c", bufs=1) as pcc, \
             tc.tile_pool(name="phC_s", bufs=4) as pcs, \
             tc.tile_pool(name="psC", bufs=2, space="PSUM") as psC, \
             tc.tile_pool(name="psCT", bufs=2, space="PSUM") as psCT:
            wo_sb = pcc.tile([P, D // P, D], BF16)
            wo_r = wo.ap().rearrange("(ko p) n -> p ko n", p=P)
            for ki in range(D // P):
                nc.sync.dma_start(out=wo_sb[:, ki], in_=wo_r[:, ki])
            g1_bc = pcc.tile([P, D], F32)
            be1_bc = pcc.tile([P, D], F32)
            nc.gpsimd.dma_start(out=g1_bc, in_=bcast(g1, D))
            nc.gpsimd.dma_start(out=be1_bc, in_=bcast(be1, D))
            xq_r = xq.ap().rearrange("(qm p) d -> p qm d", p=P)
            for qm in range(TQ // P):
                xq_t = pcs.tile([P, D], F32, tag="xqt", name="xqt")
                nc.sync.dma_start(out=xq_t, in_=xq_r[:, qm])
                pss = [psC.tile([P, 512], F32, tag=f"psC{n}", name=f"psC{n}")
                       for n in range(2)]
                for ki in range(D // P):
                    for n in range(2):
                        nc.tensor.matmul(
                            pss[n], ctxT[:, ki, qm * P:(qm + 1) * P],
                            wo_sb[:, ki, n * 512:(n + 1) * 512],
                            start=(ki == 0), stop=(ki == D // P - 1))
                for n in range(2):
                    nc.vector.scalar_tensor_tensor(
                        out=out1[:, qm, n * 512:(n + 1) * 512], in0=pss[n],
                        scalar=1.0, op0=ALU.bypass, op1=ALU.add,
                        in1=xq_t[:, n * 512:(n + 1) * 512])
                _layernorm(nc, pcs, out1[:, qm], g1_bc, be1_bc, eps_t)

            for qm in range(TQ // P):
                for dm in range(D // P):
                    tp = psCT.tile([P, P], F32, tag="tp", name="tp")
                    nc.tensor.transpose(tp, out1[:, qm, dm * P:(dm + 1) * P], ident)
                    nc.vector.tensor_copy(
                        out=out1T[:, dm, qm * P:(qm + 1) * P], in_=tp)

        ctx_pool.release()

        # ================= FFN + LN2 =================
        with tc.tile_pool(name="phD_c", bufs=1) as pdc, \
             tc.tile_pool(name="phD_h", bufs=1) as pdh, \
             tc.tile_pool(name="phD_w", bufs=4) as pdw, \
             tc.tile_pool(name="phD_s", bufs=4) as pds, \
             tc.tile_pool(name="psD1", bufs=3, space="PSUM") as psD1, \
             tc.tile_pool(name="psD2", bufs=1, space="PSUM") as psD2:
            g2_bc = pdc.tile([P, D], F32)
            be2_bc = pdc.tile([P, D], F32)
            nc.gpsimd.dma_start(out=g2_bc, in_=bcast(g2, D))
            nc.gpsimd.dma_start(out=be2_bc, in_=bcast(be2, D))
            w2_r = w2.ap().rearrange("(ko p) n -> p ko n", p=P)
            for qc in range(TQ // 512):
                hT = pdh.tile([P, DFF // P, 512], F32R, tag="hT", name="hT")
                for dm in range(DFF // P):
                    w1t = pdw.tile([P, D // P, P], F32R, tag="w1t", name="w1t")
                    nc.sync.dma_start(out=w1t, in_=w1p.ap()[dm])
                    ps = psD1.tile([P, 512], F32, tag="ps1", name="ps1")
                    for ki in range(D // P):
                        nc.tensor.matmul(
                            ps, w1t[:, ki],
                            out1T[:, ki, qc * 512:(qc + 1) * 512],
                            start=(ki == 0), stop=(ki == D // P - 1))
                    nc.scalar.activation(out=hT[:, dm], in_=ps, func=AF.Relu,
                                         bias=b1_sb[:, dm:dm + 1])
                for n in range(2):
                    pss = [psD2.tile([P, 512], F32, tag=f"ps2_{i}", name=f"ps2_{i}")
                           for i in range(4)]
                    for ds_ in range(DFF // P):
                        w2t = pdw.tile([P, 512], F32R, tag="w2t", name="w2t")
                        deng = nc.sync if ds_ % 2 == 0 else nc.scalar
                        deng.dma_start(
                            out=w2t, in_=w2_r[:, ds_, n * 512:(n + 1) * 512])
                        for j in range(4):
                            nc.tensor.matmul(
                                pss[j], hT[:, ds_, j * P:(j + 1) * P],
                                w2t, start=(ds_ == 0), stop=False)
                    for j in range(4):
                        nc.tensor.matmul(pss[j], ones_row,
                                         b2_sb[:, n * 512:(n + 1) * 512],
                                         start=False, stop=True)
                        qm = qc * 4 + j
                        nc.vector.scalar_tensor_tensor(
                            out=out1[:, qm, n * 512:(n + 1) * 512], in0=pss[j],
                            scalar=1.0, op0=ALU.bypass, op1=ALU.add,
                            in1=out1[:, qm, n * 512:(n + 1) * 512])
                for j in range(4):
                    qm = qc * 4 + j
                    _layernorm(nc, pds, out1[:, qm], g2_bc, be2_bc, eps_t)
                    nc.sync.dma_start(
                        out=out.ap().rearrange("(qm p) d -> p qm d", p=P)[:, qm],
                        in_=out1[:, qm])

        ffnin.release()
        consts.release()

    nc.compile()
    return nc


def _layernorm(nc, pool, x_ap, g_bc, be_bc, eps_t):
    """In-place LN over the free dim of x_ap [128, D]."""
    stats = pool.tile([P, D // 512, 6], F32, tag="ln_stats", name="ln_stats")
    mv = pool.tile([P, 2], F32, tag="ln_mv", name="ln_mv")
    xg = x_ap.rearrange("p (s f) -> p s f", f=512)
    for s in range(D // 512):
        nc.vector.bn_stats(out=stats[:, s], in_=xg[:, s])
    nc.vector.bn_aggr(out=mv, in_=stats)
    rstd = pool.tile([P, 1], F32, tag="ln_rstd", name="ln_rstd")
    nc.scalar.activation(out=rstd, in_=mv[:, 1:2], func=AF.Sqrt, bias=eps_t)
    nc.vector.reciprocal(out=rstd, in_=rstd)
    nc.vector.tensor_scalar(out=x_ap, in0=x_ap, scalar1=mv[:, 0:1],
                            scalar2=rstd, op0=ALU.subtract, op1=ALU.mult)
    nc.gpsimd.tensor_tensor(out=x_ap, in0=x_ap, in1=g_bc, op=ALU.mult)
    nc.gpsimd.tensor_tensor(out=x_ap, in0=x_ap, in1=be_bc, op=ALU.add)


_NC_CACHE = None


def _get_nc():
    global _NC_CACHE
    if _NC_CACHE is None:
        _NC_CACHE = build()
    return _NC_CACHE


def _prep_in_maps(inputs):
    x = np.asarray(inputs["x"], dtype=np.float32)        # [4, 2048, 1024]
    mask = np.asarray(inputs["mask"], dtype=np.float32)  # [4, 1, 1, 2048]
    bf = ml_dtypes.bfloat16
    w = {k: np.asarray(inputs[k], dtype=np.float32) for k in
         ("wq", "bq", "wk", "bk", "wv", "bv", "wo", "bo", "w1", "b1",
          "w2", "b2", "g1", "beta1", "g2", "beta2")}

    # w1 packed so each [128p, 8ko, 128n] tile is per-partition contiguous
    w1p = np.ascontiguousarray(
        w["w1"].reshape(D // P, P, DFF // P, P).transpose(2, 1, 0, 3))
    shared = {
        "wq": w["wq"].astype(bf), "wk": w["wk"].astype(bf), "wv": w["wv"].astype(bf),
        "bq": w["bq"], "bk": w["bk"], "bv": w["bv"],
        "wo": w["wo"].astype(bf), "w1p": w1p, "b1": w["b1"],
        "w2": w["w2"], "b2": w["b2"],
        "g1": w["g1"], "be1": w["beta1"], "g2": w["g2"], "be2": w["beta2"],
    }
    in_maps = []
    for c in range(8):
        b, half = c // 2, c % 2
        m = dict(shared)
        xb = x[b]
        order = np.r_[half * TQ:(half + 1) * TQ, (1 - half) * TQ:(2 - half) * TQ]
        m["xt"] = np.ascontiguousarray(xb[order].T).astype(bf)
        m["xq"] = np.ascontiguousarray(
            xb[half * TQ:(half + 1) * TQ] + w["bo"][None, :])
        m["maskb"] = np.ascontiguousarray(mask[b, 0, 0][order])
        in_maps.append(m)
    return in_maps


def kernel(**inputs):
    in_maps = _prep_in_maps(inputs)
    nc = _get_nc()
    res = run_bass_kernel_spmd(nc, in_maps, core_ids=list(range(8)))
    outp = np.empty((4, 2048, 1024), dtype=np.float32)
    for c in range(8):
        b, half = c // 2, c % 2
        outp[b, half * TQ:(half + 1) * TQ] = res.results[c]["out"]
    return outp
